# revision 15
# baseline (speedup 1.0000x reference)
"""Trainium2 Bass kernel for nn_Classifier (capsule conv + routing), v2.

Math (validated vs jax reference; fp16 operands give ~3e-3 rel err):
  W = conv_w[:,0,:]                                    # [16, 640]
  y[b,i,o]   = relu(sum_t x[b,i,t] W[t,o] + conv_b[o])
  U[b,k,i,d] = y[b,i,k*64+d]
  Usum[b,k,d]= sum_i U[b,k,i,d]
  logits     = (U . Usum)/4 -> softmax over i -> C;  Cb = C + B_bias
  S[b,k,:]   = sum_i Cb[b,k,i] U[b,k,i,:]
  out[b,k]   = n2/(n2+1),  n2 = |S|^2   (eps/sqrt factor ~1e-7, dropped)

Design notes (v2, ~3x faster than v1):
  - All matmul operands fp16 (1 col/cycle PE stream vs 2 for f32r; FWL
    weight loads; 16-bit DVE packing on SBUF ops). fp16 keeps 10 mantissa
    bits -> 3e-3 end-to-end vs 1.7e-2 for bf16 (softmax logits ~50-120).
  - Host pre-packs x into PE row-band layout and all constant masks, so
    the kernel has zero on-chip preamble (no iota/affine_select/PE input
    transposes): 2 input DMAs total.
  - Conv computed in both orientations on PE (contraction=17 row-banded
    4x via tile_position). PSUM->SBUF evictions (the real bottleneck:
    f32 PSUM reads run 1 elem/cycle) alternate DVE/ACT; relu and the
    usum row-reduction are fused into the eviction (accum_out).
  - gmat (usum-scaled logit weights) built on GpSimd (idle otherwise).
  - Single PSUM scope, 8 banks via tag reuse, so conv(g1) overlaps
    routing(g0) and the PE never idles long enough to lose HAM warmup.
  - Tail: out = n2/(n2+1) on [128,2], PE-transposed to [2,128] and
    stored with one 512B DMA (v1 used 8 scattered 40B DMAs, ~10us).

Per-core layout (8 batches/core, b = 4g+j, g in {0,1}, j in 0..3):
  xT[g]  [128,512] rows 32j+t = x[b,i,t], row 32j+16 = 1.0 (bias fold)
  w4     [128,640] rows 32j+t = W[t,o], row 32j+16 = conv_b
  yr_oi[b] [128,5,512]  chunk c: y[o=c*128+p, i]      (fp16)
  yr_io[g] [128,4,4,640] [p, j, q, o]: y[i=q*128+p, o] (fp16)
  logits via G matmul (G[o,k'] = 0.25*usum[o]*[class(o)==k']), softmax
  with per-row max, Cb = exp/Z + B; CbT via PE transpose; S col-tiled.
"""

import numpy as np

import concourse.bass as bass
import concourse.mybir as mybir
import concourse.tile as tile
from concourse import bacc
from concourse.bass_utils import run_bass_kernel_spmd

F32 = mybir.dt.float32
F16 = mybir.dt.float16
BF16 = mybir.dt.bfloat16

B_FULL = 64
N = 512          # num timecaps (routing dim i)
DT = 16          # dim timecaps (conv contraction)
K = 10           # classes
D = 64           # dim classes
NO = K * D       # 640 conv output channels
NCORES = 8
BPC = B_FULL // NCORES   # 8 batches per core

# const-block column offsets
C_XT0, C_XT1, C_W4, C_BB, C_GM, C_SM, C_ID = 0, 512, 1024, 1664, 2176, 2226, 2866
C_TOT = 2994


def _build_program():
    nc = bacc.Bacc("TRN2", target_bir_lowering=False)
    cst_in = nc.declare_dram_parameter("cst", [128, C_TOT], F16, isOutput=False)
    out_d = nc.declare_dram_parameter("out", [2, 128], F16, isOutput=True)

    AF = mybir.ActivationFunctionType
    OP = mybir.AluOpType

    with tile.TileContext(nc) as tc:
        with tc.tile_pool(name="const", bufs=1) as cpool:
            cst = cpool.tile([128, C_TOT], F16, name="cst", tag="cst")
            dummy = cpool.tile([128, 1], F32, name="dummy", tag="dummy")
            dwarm = cpool.tile([128, N], F16, name="dwarm", tag="dwarm")
            # load x+w first so conv can start; rest lands during conv
            nc.sync.dma_start(cst[:, 0:C_BB], cst_in[:, 0:C_BB])
            nc.sync.dma_start(cst[:, C_BB:C_TOT], cst_in[:, C_BB:C_TOT])
            # prefetch the exp table set (relu/copy ride along in every set)
            nc.vector.memset(dummy[:], 0.0)
            nc.scalar.activation(out=dummy[:], in_=dummy[:], func=AF.Exp)
            nc.gpsimd.memset(dwarm[:], 0.0)

            xT = [cst[:, C_XT0:C_XT0 + N], cst[:, C_XT1:C_XT1 + N]]
            w4 = cst[:, C_W4:C_W4 + NO]
            bb = cst[:, C_BB:C_BB + N]
            gm = cst[:, C_GM:C_GM + 5 * K]
            sm = cst[:, C_SM:C_SM + NO]
            idt = cst[:, C_ID:C_ID + 128]

            with tc.tile_pool(name="data", bufs=1) as dpool:
                yr_oi = [dpool.tile([128, 5, N], F16, name=f"yroi{b}", tag=f"yroi{b}")
                         for b in range(BPC)]
                yr_io = [dpool.tile([128, 4, 4, NO], BF16, name=f"yrio{g}", tag=f"yrio{g}")
                         for g in range(2)]
                usum = [dpool.tile([128, 4, 5], F32, name=f"us{g}", tag=f"us{g}")
                        for g in range(2)]
                gmat = [dpool.tile([128, 4, 5, K], F16, name=f"gmat{g}", tag=f"gmat{g}")
                        for g in range(2)]
                exp_sb = [dpool.tile([128, N], F16, name=f"exp{g}", tag=f"exp{g}")
                          for g in range(2)]
                cb_sb = [dpool.tile([128, N], F16, name=f"cb{g}", tag=f"cb{g}")
                         for g in range(2)]
                negmax = [dpool.tile([128, 1], F32, name=f"nm{g}", tag=f"nm{g}")
                          for g in range(2)]
                zsum = [dpool.tile([128, 1], F32, name=f"z{g}", tag=f"z{g}")
                        for g in range(2)]
                rz = [dpool.tile([128, 1], F32, name=f"rz{g}", tag=f"rz{g}")
                      for g in range(2)]
                ebt = [dpool.tile([128, 4, 4, K], BF16, name=f"ebt{g}", tag=f"ebt{g}")
                       for g in range(2)]
                smm = [dpool.tile([128, NO], F16, name=f"smm{g}", tag=f"smm{g}")
                       for g in range(2)]
                sqs = [dpool.tile([128, NO], F16, name=f"sqs{g}", tag=f"sqs{g}")
                       for g in range(2)]
                n2 = dpool.tile([128, 2], F32, name="n2", tag="n2")
                t_a = dpool.tile([128, 2], F32, name="t_a", tag="t_a")
                t_d = dpool.tile([128, 2], F32, name="t_d", tag="t_d")
                t_b = dpool.tile([128, 2], F16, name="t_b", tag="t_b")
                out_sb = dpool.tile([2, 128], F16, name="out_sb", tag="out_sb")

                evict_i = 0

                def evict(dst, src, acc=None):
                    # PSUM->SBUF relu eviction, alternating DVE/ACT
                    nonlocal evict_i
                    if evict_i % 2 == 0:
                        if acc is not None:
                            nc.vector.tensor_scalar(
                                out=dst, in0=src, scalar1=0.0, scalar2=0.0,
                                op0=OP.max, op1=OP.add, accum_out=acc)
                        else:
                            nc.vector.tensor_scalar(
                                out=dst, in0=src, scalar1=0.0, scalar2=None,
                                op0=OP.max)
                    else:
                        nc.scalar.activation(
                            out=dst, in_=src, func=AF.Relu, accum_out=acc)
                    evict_i += 1

                with tc.tile_pool(name="ps", bufs=1, space="PSUM") as pp:
                    lg = [None, None]
                    ps_sm = [None, None]
                    ps_sr = [None, None]

                    # HAM warm-up: keep the PE array streaming during the
                    # input-DMA dead time so the clock gate opens (4/8 ->
                    # 8/8) before real matmuls start. WAW on one bank
                    # serializes them back-to-back; results never read.
                    ps_w = pp.tile([128, N], F32, name="warm", tag="lg",
                                   bufs=2)
                    for _ in range(26):
                        nc.tensor.matmul(
                            ps_w[:], dwarm[0:DT + 1, 0:128],
                            dwarm[0:DT + 1, :],
                            start=True, stop=True)

                    def conv_oi(g, cs):
                        for c in cs:
                            for j in range(4):
                                b = 4 * g + j
                                ps = pp.tile([128, N], F32, name="oi",
                                             tag="conv", bufs=5)
                                nc.tensor.matmul(
                                    ps[:],
                                    w4[32 * j:32 * j + DT + 1,
                                       c * 128:(c + 1) * 128],
                                    xT[g][32 * j:32 * j + DT + 1, :],
                                    start=True, stop=True,
                                    tile_position=(32 * j, 0))
                                evict(yr_oi[b][:, c, :], ps[:],
                                      usum[g][:, j, c:c + 1])
                                # G chunk on gpsimd (idle engine)
                                nc.gpsimd.tensor_scalar(
                                    out=gmat[g][:, j, c, :],
                                    in0=gm[:, c * K:(c + 1) * K],
                                    scalar1=usum[g][:, j, c:c + 1],
                                    scalar2=None, op0=OP.mult)

                    def conv_io(g, js):
                        for j in js:
                            b = 4 * g + j
                            rem = pp.tile([128, 4, 128], F32, name="rem",
                                          tag="rem", bufs=1)
                            for q in range(4):
                                ps = pp.tile([128, N], F32, name="iom",
                                             tag="conv", bufs=5)
                                nc.tensor.matmul(
                                    ps[:],
                                    xT[g][32 * j:32 * j + DT + 1,
                                          q * 128:(q + 1) * 128],
                                    w4[32 * j:32 * j + DT + 1, 0:N],
                                    start=True, stop=True,
                                    tile_position=(32 * j, 0))
                                nc.tensor.matmul(
                                    rem[:, q, :],
                                    xT[g][32 * j:32 * j + DT + 1,
                                          q * 128:(q + 1) * 128],
                                    w4[32 * j:32 * j + DT + 1, N:NO],
                                    start=True, stop=True,
                                    tile_position=(32 * j, 0))
                                evict(yr_io[g][:, j, q, 0:N], ps[:])
                            evict(yr_io[g][:, j, :, N:NO], rem[:])

                    def logits(g):
                        lg[g] = pp.tile([128, N], F32, name="lg",
                                        tag="lg", bufs=2)
                        for c in range(5):
                            for j in range(4):
                                b = 4 * g + j
                                nc.tensor.matmul(
                                    lg[g][32 * j:32 * j + K, :],
                                    gmat[g][:, j, c, :],
                                    yr_oi[b][:, c, :],
                                    start=(c == 0), stop=(c == 4),
                                    tile_position=(0, 32 * j))

                    def softmax(g):
                        nc.vector.tensor_reduce(
                            out=negmax[g][:], in_=lg[g][:],
                            op=OP.max, axis=mybir.AxisListType.X, negate=True)
                        nc.scalar.activation(
                            out=exp_sb[g][:], in_=lg[g][:], func=AF.Exp,
                            bias=negmax[g][:], scale=1.0,
                            accum_out=zsum[g][:])
                        nc.vector.reciprocal(rz[g][:], zsum[g][:])
                        nc.vector.scalar_tensor_tensor(
                            out=cb_sb[g][:], in0=exp_sb[g][:],
                            scalar=rz[g][:], in1=bb[:],
                            op0=OP.mult, op1=OP.add)

                    def cb_transpose(g):
                        cbt = pp.tile([128, 4, 128], F16, name="cbt",
                                      tag="lg", bufs=2)
                        for q in range(4):
                            nc.tensor.transpose(
                                cbt[:, q, :],
                                cb_sb[g][:, q * 128:(q + 1) * 128],
                                idt[:])
                        # transposed cols are 32j+k' -> pick each j's K cols
                        nc.vector.tensor_copy(
                            ebt[g][:],
                            cbt[:].rearrange("p q (a w) -> p q a w", a=4)[
                                :, :, :, 0:K])

                    def s_matmuls(g):
                        ps_sm[g] = pp.tile([128, N], F32, name="sm",
                                           tag="conv", bufs=5)
                        ps_sr[g] = pp.tile([128, 128], F32, name="sr",
                                           tag="rem", bufs=1)
                        for q in range(4):
                            for j in range(4):
                                nc.tensor.matmul(
                                    ps_sm[g][32 * j:32 * j + K, :],
                                    ebt[g][:, q, j, :],
                                    yr_io[g][:, j, q, 0:N],
                                    start=(q == 0), stop=(q == 3),
                                    tile_position=(0, 32 * j))
                                nc.tensor.matmul(
                                    ps_sr[g][32 * j:32 * j + K, :],
                                    ebt[g][:, q, j, :],
                                    yr_io[g][:, j, q, N:NO],
                                    start=(q == 0), stop=(q == 3),
                                    tile_position=(0, 32 * j))

                    def s_norm(g):
                        nc.vector.tensor_tensor(
                            out=smm[g][:, 0:N], in0=ps_sm[g][:],
                            in1=sm[:, 0:N], op=OP.mult)
                        nc.vector.tensor_tensor(
                            out=smm[g][:, N:NO], in0=ps_sr[g][:],
                            in1=sm[:, N:NO], op=OP.mult)
                        nc.vector.scalar_tensor_tensor(
                            out=sqs[g][:], in0=smm[g][:],
                            scalar=1.0, in1=smm[g][:],
                            op0=OP.mult, op1=OP.mult,
                            accum_out=n2[:, g:g + 1])

    # ---- phase schedule (engine queues are FIFO; order = schedule) ----
                    conv_oi(0, range(5))
                    logits(0)
                    conv_oi(1, [0, 1, 2])
                    softmax(0)
                    cb_transpose(0)
                    conv_oi(1, [3, 4])
                    conv_io(0, range(4))
                    conv_io(1, [0])
                    s_matmuls(0)
                    conv_io(1, [1])
                    logits(1)
                    s_norm(0)
                    conv_io(1, [2])
                    softmax(1)
                    cb_transpose(1)
                    conv_io(1, [3])
                    s_matmuls(1)
                    s_norm(1)

                    # ---- squash tail: out = n2/(n2+1), transposed store
                    nc.vector.tensor_scalar(
                        out=t_a[:], in0=n2[:], scalar1=1.0, scalar2=None,
                        op0=OP.add)
                    nc.vector.reciprocal(t_d[:], t_a[:])
                    nc.vector.tensor_tensor(
                        out=t_b[:], in0=n2[:], in1=t_d[:], op=OP.mult)
                    outT = pp.tile([2, 128], F16, name="outT",
                                   tag="lg", bufs=2)
                    nc.tensor.transpose(outT[:], t_b[:], idt[:])
                    nc.vector.tensor_copy(out_sb[:], outT[:])
                    nc.sync.dma_start(out_d[:, :], out_sb[:])
    nc.compile()
    return nc


_PROGRAM_CACHE = None


def _get_program():
    global _PROGRAM_CACHE
    if _PROGRAM_CACHE is None:
        _PROGRAM_CACHE = _build_program()
    return _PROGRAM_CACHE


def _build_const_common():
    """Constant part of the cst block (cols C_W4..end), batch-independent."""
    blk = np.zeros((128, C_TOT), dtype=np.float16)
    jj = np.arange(4)
    # gm: [p, c*10+k'] = 0.25 if k' == 2c + p//64
    p = np.arange(128)
    for c in range(5):
        for kp in range(K):
            blk[:, C_GM + c * K + kp] = np.where(2 * c + p // 64 == kp, 0.25, 0.0)
    # sm: rows 32j+k', cols [64k',64(k'+1)) = 1
    for j in range(4):
        for kp in range(K):
            blk[32 * j + kp, C_SM + D * kp:C_SM + D * (kp + 1)] = 1.0
    # identity
    blk[:, C_ID:C_ID + 128] = np.eye(128, dtype=np.float16)
    return blk


_CONST_COMMON = None


def build_in_maps(timecaps, conv_w, conv_b, B_bias):
    global _CONST_COMMON
    timecaps = np.asarray(timecaps, dtype=np.float32)
    conv_w = np.asarray(conv_w, dtype=np.float32)
    conv_b = np.asarray(conv_b, dtype=np.float32)
    B_bias = np.asarray(B_bias, dtype=np.float32)

    if _CONST_COMMON is None:
        _CONST_COMMON = _build_const_common()
    base = _CONST_COMMON.copy()
    W = conv_w[:, 0, :].astype(np.float16)         # [16, 640]
    cb16 = conv_b.astype(np.float16)
    bb16 = B_bias[:, 0, :].astype(np.float16)      # [10, 512]
    for j in range(4):
        base[32 * j:32 * j + DT, C_W4:C_W4 + NO] = W
        base[32 * j + DT, C_W4:C_W4 + NO] = cb16
        base[32 * j:32 * j + K, C_BB:C_BB + N] = bb16

    # x -> [core, g, j, t, i] fp16 row-band layout
    xt = timecaps.astype(np.float16).transpose(0, 2, 1)   # [64, 16, 512]
    xt = xt.reshape(NCORES, 2, 4, DT, N)

    in_maps = []
    for core in range(NCORES):
        cst = base.copy()
        for g in range(2):
            col = C_XT0 if g == 0 else C_XT1
            for j in range(4):
                cst[32 * j:32 * j + DT, col:col + N] = xt[core, g, j]
                cst[32 * j + DT, col:col + N] = 1.0
        in_maps.append({"cst": cst})
    return in_maps


def assemble_out(res):
    out = np.zeros((B_FULL, K, 1), dtype=np.float32)
    for core in range(NCORES):
        r = np.asarray(res.results[core]["out"], dtype=np.float32)  # [2, 128]
        for g in range(2):
            for j in range(4):
                out[core * BPC + 4 * g + j, :, 0] = r[g, 32 * j:32 * j + K]
    return out


def kernel(timecaps, conv_w, conv_b, B_bias):
    in_maps = build_in_maps(timecaps, conv_w, conv_b, B_bias)
    nc = _get_program()
    res = run_bass_kernel_spmd(nc, in_maps, list(range(NCORES)))
    return assemble_out(res)


if __name__ == "__main__":
    rng = np.random.default_rng(0)
    ins = {
        "timecaps": rng.standard_normal((B_FULL, N, DT), dtype=np.float32),
        "conv_w": (rng.standard_normal((DT, 1, NO), dtype=np.float32) * 0.05),
        "conv_b": np.zeros((NO,), dtype=np.float32),
        "B_bias": (rng.standard_normal((K, 1, N), dtype=np.float32) * 0.05),
    }
    print(kernel(**ins)[:2, :, 0])


# revision 16
# speedup vs baseline: 1.3212x; 1.3212x over previous
"""Trainium2 Bass kernel for nn_Classifier (capsule conv + routing), v2.

Math (validated vs jax reference; fp16 operands give ~3e-3 rel err):
  W = conv_w[:,0,:]                                    # [16, 640]
  y[b,i,o]   = relu(sum_t x[b,i,t] W[t,o] + conv_b[o])
  U[b,k,i,d] = y[b,i,k*64+d]
  Usum[b,k,d]= sum_i U[b,k,i,d]
  logits     = (U . Usum)/4 -> softmax over i -> C;  Cb = C + B_bias
  S[b,k,:]   = sum_i Cb[b,k,i] U[b,k,i,:]
  out[b,k]   = n2/(n2+1),  n2 = |S|^2   (eps/sqrt factor ~1e-7, dropped)

Design notes (v2, ~3x faster than v1):
  - All matmul operands fp16 (1 col/cycle PE stream vs 2 for f32r; FWL
    weight loads; 16-bit DVE packing on SBUF ops). fp16 keeps 10 mantissa
    bits -> 3e-3 end-to-end vs 1.7e-2 for bf16 (softmax logits ~50-120).
  - Host pre-packs x into PE row-band layout and all constant masks, so
    the kernel has zero on-chip preamble (no iota/affine_select/PE input
    transposes): 2 input DMAs total.
  - Conv computed in both orientations on PE (contraction=17 row-banded
    4x via tile_position). PSUM->SBUF evictions (the real bottleneck:
    f32 PSUM reads run 1 elem/cycle) alternate DVE/ACT; relu and the
    usum row-reduction are fused into the eviction (accum_out).
  - gmat (usum-scaled logit weights) built on GpSimd (idle otherwise).
  - Single PSUM scope, 8 banks via tag reuse, so conv(g1) overlaps
    routing(g0) and the PE never idles long enough to lose HAM warmup.
  - Tail: out = n2/(n2+1) on [128,2], PE-transposed to [2,128] and
    stored with one 512B DMA (v1 used 8 scattered 40B DMAs, ~10us).

Per-core layout (8 batches/core, b = 4g+j, g in {0,1}, j in 0..3):
  xT[g]  [128,512] rows 32j+t = x[b,i,t], row 32j+16 = 1.0 (bias fold)
  w4     [128,640] rows 32j+t = W[t,o], row 32j+16 = conv_b
  yr_oi[b] [128,5,512]  chunk c: y[o=c*128+p, i]      (fp16)
  yr_io[g] [128,4,4,640] [p, j, q, o]: y[i=q*128+p, o] (fp16)
  logits via G matmul (G[o,k'] = 0.25*usum[o]*[class(o)==k']), softmax
  with per-row max, Cb = exp/Z + B; CbT via PE transpose; S col-tiled.
"""

import numpy as np

import concourse.bass as bass
import concourse.mybir as mybir
import concourse.tile as tile
from concourse import bacc
from concourse.bass_utils import run_bass_kernel_spmd

F32 = mybir.dt.float32
F16 = mybir.dt.float16
BF16 = mybir.dt.bfloat16

B_FULL = 64
N = 512          # num timecaps (routing dim i)
DT = 16          # dim timecaps (conv contraction)
K = 10           # classes
D = 64           # dim classes
NO = K * D       # 640 conv output channels
NCORES = 8
BPC = B_FULL // NCORES   # 8 batches per core

# const-block column offsets
C_XT0, C_XT1, C_W4, C_BB, C_GM, C_SM, C_ID = 0, 512, 1024, 1664, 2176, 2226, 2866
C_TOT = 2994


def _build_program():
    nc = bacc.Bacc("TRN2", target_bir_lowering=False)
    cst_in = nc.declare_dram_parameter("cst", [128, C_TOT], F16, isOutput=False)
    out_d = nc.declare_dram_parameter("out", [2, 128], F16, isOutput=True)

    AF = mybir.ActivationFunctionType
    OP = mybir.AluOpType

    with tile.TileContext(nc) as tc:
        with tc.tile_pool(name="const", bufs=1) as cpool:
            cst = cpool.tile([128, C_TOT], F16, name="cst", tag="cst")
            dummy = cpool.tile([128, 1], F32, name="dummy", tag="dummy")
            dwarm = cpool.tile([128, N], F16, name="dwarm", tag="dwarm")
            # load x+w first so conv can start; rest lands during conv
            nc.sync.dma_start(cst[:, 0:C_BB], cst_in[:, 0:C_BB])
            nc.sync.dma_start(cst[:, C_BB:C_TOT], cst_in[:, C_BB:C_TOT])
            # prefetch the exp table set (relu/copy ride along in every set)
            nc.vector.memset(dwarm[:], 0.0)
            nc.vector.memset(dummy[:], 0.0)
            nc.scalar.activation(out=dummy[:], in_=dummy[:], func=AF.Exp)

            xT = [cst[:, C_XT0:C_XT0 + N], cst[:, C_XT1:C_XT1 + N]]
            w4 = cst[:, C_W4:C_W4 + NO]
            bb = cst[:, C_BB:C_BB + N]
            gm = cst[:, C_GM:C_GM + 5 * K]
            sm = cst[:, C_SM:C_SM + NO]
            idt = cst[:, C_ID:C_ID + 128]

            with tc.tile_pool(name="data", bufs=1) as dpool:
                yr_oi = [dpool.tile([128, 5, N], F16, name=f"yroi{b}", tag=f"yroi{b}")
                         for b in range(BPC)]
                yr_io = [dpool.tile([128, 4, 4, NO], BF16, name=f"yrio{g}", tag=f"yrio{g}")
                         for g in range(2)]
                usum = [dpool.tile([128, 4, 5], F32, name=f"us{g}", tag=f"us{g}")
                        for g in range(2)]
                gmat = [dpool.tile([128, 4, 5, K], F16, name=f"gmat{g}", tag=f"gmat{g}")
                        for g in range(2)]
                exp_sb = [dpool.tile([128, N], F16, name=f"exp{g}", tag=f"exp{g}")
                          for g in range(2)]
                cb_sb = [dpool.tile([128, N], F16, name=f"cb{g}", tag=f"cb{g}")
                         for g in range(2)]
                negmax = [dpool.tile([128, 1], F32, name=f"nm{g}", tag=f"nm{g}")
                          for g in range(2)]
                zsum = [dpool.tile([128, 1], F32, name=f"z{g}", tag=f"z{g}")
                        for g in range(2)]
                rz = [dpool.tile([128, 1], F32, name=f"rz{g}", tag=f"rz{g}")
                      for g in range(2)]
                ebt = [dpool.tile([128, 4, 4, K], BF16, name=f"ebt{g}", tag=f"ebt{g}")
                       for g in range(2)]
                smm = [dpool.tile([128, NO], F16, name=f"smm{g}", tag=f"smm{g}")
                       for g in range(2)]
                sqs = [dpool.tile([128, NO], F16, name=f"sqs{g}", tag=f"sqs{g}")
                       for g in range(2)]
                n2 = dpool.tile([128, 2], F32, name="n2", tag="n2")
                t_a = dpool.tile([128, 2], F32, name="t_a", tag="t_a")
                t_d = dpool.tile([128, 2], F32, name="t_d", tag="t_d")
                t_b = dpool.tile([128, 2], F16, name="t_b", tag="t_b")
                out_sb = dpool.tile([2, 128], F16, name="out_sb", tag="out_sb")

                evict_i = 0

                def evict(dst, src, acc=None):
                    # PSUM->SBUF relu eviction, alternating DVE/ACT
                    nonlocal evict_i
                    if evict_i % 2 == 0:
                        if acc is not None:
                            nc.vector.tensor_scalar(
                                out=dst, in0=src, scalar1=0.0, scalar2=0.0,
                                op0=OP.max, op1=OP.add, accum_out=acc)
                        else:
                            nc.vector.tensor_scalar(
                                out=dst, in0=src, scalar1=0.0, scalar2=None,
                                op0=OP.max)
                    else:
                        nc.scalar.activation(
                            out=dst, in_=src, func=AF.Relu, accum_out=acc)
                    evict_i += 1

                with tc.tile_pool(name="ps", bufs=1, space="PSUM") as pp:
                    lg = [None, None]
                    ps_sm = [None, None]
                    ps_sr = [None, None]

                    # HAM warm-up: stream on all 4 row groups during the
                    # input-DMA dead time so the clock gate opens (4/8 ->
                    # 8/8) before real matmuls start; results never read.
                    for _ in range(9):
                        for w in range(4):
                            ps_w = pp.tile([128, N], F32, name="warm",
                                           tag="conv", bufs=5)
                            nc.tensor.matmul(
                                ps_w[:],
                                dwarm[32 * w:32 * w + DT + 1, 0:128],
                                dwarm[32 * w:32 * w + DT + 1, :],
                                start=True, stop=True,
                                tile_position=(32 * w, 0))

                    def conv_oi(g, cs):
                        for c in cs:
                            for j in range(4):
                                b = 4 * g + j
                                ps = pp.tile([128, N], F32, name="oi",
                                             tag="conv", bufs=5)
                                nc.tensor.matmul(
                                    ps[:],
                                    w4[32 * j:32 * j + DT + 1,
                                       c * 128:(c + 1) * 128],
                                    xT[g][32 * j:32 * j + DT + 1, :],
                                    start=True, stop=True,
                                    tile_position=(32 * j, 0))
                                evict(yr_oi[b][:, c, :], ps[:],
                                      usum[g][:, j, c:c + 1])
                                # G chunk on gpsimd (idle engine)
                                nc.gpsimd.tensor_scalar(
                                    out=gmat[g][:, j, c, :],
                                    in0=gm[:, c * K:(c + 1) * K],
                                    scalar1=usum[g][:, j, c:c + 1],
                                    scalar2=None, op0=OP.mult)

                    def conv_io(g, js):
                        for j in js:
                            b = 4 * g + j
                            rem = pp.tile([128, 4, 128], F32, name="rem",
                                          tag="rem", bufs=1)
                            for q in range(4):
                                ps = pp.tile([128, N], F32, name="iom",
                                             tag="conv", bufs=5)
                                nc.tensor.matmul(
                                    ps[:],
                                    xT[g][32 * j:32 * j + DT + 1,
                                          q * 128:(q + 1) * 128],
                                    w4[32 * j:32 * j + DT + 1, 0:N],
                                    start=True, stop=True,
                                    tile_position=(32 * j, 0))
                                nc.tensor.matmul(
                                    rem[:, q, :],
                                    xT[g][32 * j:32 * j + DT + 1,
                                          q * 128:(q + 1) * 128],
                                    w4[32 * j:32 * j + DT + 1, N:NO],
                                    start=True, stop=True,
                                    tile_position=(32 * j, 0))
                                evict(yr_io[g][:, j, q, 0:N], ps[:])
                            evict(yr_io[g][:, j, :, N:NO], rem[:])

                    def logits(g):
                        lg[g] = pp.tile([128, N], F32, name="lg",
                                        tag="lg", bufs=2)
                        for c in range(5):
                            for j in range(4):
                                b = 4 * g + j
                                nc.tensor.matmul(
                                    lg[g][32 * j:32 * j + K, :],
                                    gmat[g][:, j, c, :],
                                    yr_oi[b][:, c, :],
                                    start=(c == 0), stop=(c == 4),
                                    tile_position=(0, 32 * j))

                    def softmax(g):
                        nc.vector.tensor_reduce(
                            out=negmax[g][:], in_=lg[g][:],
                            op=OP.max, axis=mybir.AxisListType.X, negate=True)
                        nc.scalar.activation(
                            out=exp_sb[g][:], in_=lg[g][:], func=AF.Exp,
                            bias=negmax[g][:], scale=1.0,
                            accum_out=zsum[g][:])
                        nc.vector.reciprocal(rz[g][:], zsum[g][:])
                        nc.vector.scalar_tensor_tensor(
                            out=cb_sb[g][:], in0=exp_sb[g][:],
                            scalar=rz[g][:], in1=bb[:],
                            op0=OP.mult, op1=OP.add)

                    def cb_transpose(g):
                        cbt = pp.tile([128, 4, 128], F16, name="cbt",
                                      tag="lg", bufs=2)
                        for q in range(4):
                            nc.tensor.transpose(
                                cbt[:, q, :],
                                cb_sb[g][:, q * 128:(q + 1) * 128],
                                idt[:])
                        # transposed cols are 32j+k' -> pick each j's K cols
                        nc.vector.tensor_copy(
                            ebt[g][:],
                            cbt[:].rearrange("p q (a w) -> p q a w", a=4)[
                                :, :, :, 0:K])

                    def s_matmuls(g):
                        ps_sm[g] = pp.tile([128, N], F32, name="sm",
                                           tag="conv", bufs=5)
                        ps_sr[g] = pp.tile([128, 128], F32, name="sr",
                                           tag="rem", bufs=1)
                        for q in range(4):
                            for j in range(4):
                                nc.tensor.matmul(
                                    ps_sm[g][32 * j:32 * j + K, :],
                                    ebt[g][:, q, j, :],
                                    yr_io[g][:, j, q, 0:N],
                                    start=(q == 0), stop=(q == 3),
                                    tile_position=(0, 32 * j))
                                nc.tensor.matmul(
                                    ps_sr[g][32 * j:32 * j + K, :],
                                    ebt[g][:, q, j, :],
                                    yr_io[g][:, j, q, N:NO],
                                    start=(q == 0), stop=(q == 3),
                                    tile_position=(0, 32 * j))

                    def s_norm(g):
                        nc.vector.tensor_tensor(
                            out=smm[g][:, 0:N], in0=ps_sm[g][:],
                            in1=sm[:, 0:N], op=OP.mult)
                        nc.vector.tensor_tensor(
                            out=smm[g][:, N:NO], in0=ps_sr[g][:],
                            in1=sm[:, N:NO], op=OP.mult)
                        nc.vector.scalar_tensor_tensor(
                            out=sqs[g][:], in0=smm[g][:],
                            scalar=1.0, in1=smm[g][:],
                            op0=OP.mult, op1=OP.mult,
                            accum_out=n2[:, g:g + 1])

    # ---- phase schedule (engine queues are FIFO; order = schedule) ----
                    conv_oi(0, range(5))
                    logits(0)
                    conv_oi(1, [0, 1, 2])
                    softmax(0)
                    cb_transpose(0)
                    conv_oi(1, [3, 4])
                    conv_io(0, range(4))
                    conv_io(1, [0])
                    s_matmuls(0)
                    conv_io(1, [1])
                    logits(1)
                    s_norm(0)
                    conv_io(1, [2])
                    softmax(1)
                    cb_transpose(1)
                    conv_io(1, [3])
                    s_matmuls(1)
                    s_norm(1)

                    # ---- squash tail: out = n2/(n2+1), transposed store
                    nc.vector.tensor_scalar(
                        out=t_a[:], in0=n2[:], scalar1=1.0, scalar2=None,
                        op0=OP.add)
                    nc.vector.reciprocal(t_d[:], t_a[:])
                    nc.vector.tensor_tensor(
                        out=t_b[:], in0=n2[:], in1=t_d[:], op=OP.mult)
                    outT = pp.tile([2, 128], F16, name="outT",
                                   tag="lg", bufs=2)
                    nc.tensor.transpose(outT[:], t_b[:], idt[:])
                    nc.vector.tensor_copy(out_sb[:], outT[:])
                    nc.sync.dma_start(out_d[:, :], out_sb[:])
    nc.compile()
    return nc


_PROGRAM_CACHE = None


def _get_program():
    global _PROGRAM_CACHE
    if _PROGRAM_CACHE is None:
        _PROGRAM_CACHE = _build_program()
    return _PROGRAM_CACHE


def _build_const_common():
    """Constant part of the cst block (cols C_W4..end), batch-independent."""
    blk = np.zeros((128, C_TOT), dtype=np.float16)
    jj = np.arange(4)
    # gm: [p, c*10+k'] = 0.25 if k' == 2c + p//64
    p = np.arange(128)
    for c in range(5):
        for kp in range(K):
            blk[:, C_GM + c * K + kp] = np.where(2 * c + p // 64 == kp, 0.25, 0.0)
    # sm: rows 32j+k', cols [64k',64(k'+1)) = 1
    for j in range(4):
        for kp in range(K):
            blk[32 * j + kp, C_SM + D * kp:C_SM + D * (kp + 1)] = 1.0
    # identity
    blk[:, C_ID:C_ID + 128] = np.eye(128, dtype=np.float16)
    return blk


_CONST_COMMON = None


def build_in_maps(timecaps, conv_w, conv_b, B_bias):
    global _CONST_COMMON
    timecaps = np.asarray(timecaps, dtype=np.float32)
    conv_w = np.asarray(conv_w, dtype=np.float32)
    conv_b = np.asarray(conv_b, dtype=np.float32)
    B_bias = np.asarray(B_bias, dtype=np.float32)

    if _CONST_COMMON is None:
        _CONST_COMMON = _build_const_common()
    base = _CONST_COMMON.copy()
    W = conv_w[:, 0, :].astype(np.float16)         # [16, 640]
    cb16 = conv_b.astype(np.float16)
    bb16 = B_bias[:, 0, :].astype(np.float16)      # [10, 512]
    for j in range(4):
        base[32 * j:32 * j + DT, C_W4:C_W4 + NO] = W
        base[32 * j + DT, C_W4:C_W4 + NO] = cb16
        base[32 * j:32 * j + K, C_BB:C_BB + N] = bb16

    # x -> [core, g, j, t, i] fp16 row-band layout
    xt = timecaps.astype(np.float16).transpose(0, 2, 1)   # [64, 16, 512]
    xt = xt.reshape(NCORES, 2, 4, DT, N)

    in_maps = []
    for core in range(NCORES):
        cst = base.copy()
        for g in range(2):
            col = C_XT0 if g == 0 else C_XT1
            for j in range(4):
                cst[32 * j:32 * j + DT, col:col + N] = xt[core, g, j]
                cst[32 * j + DT, col:col + N] = 1.0
        in_maps.append({"cst": cst})
    return in_maps


def assemble_out(res):
    out = np.zeros((B_FULL, K, 1), dtype=np.float32)
    for core in range(NCORES):
        r = np.asarray(res.results[core]["out"], dtype=np.float32)  # [2, 128]
        for g in range(2):
            for j in range(4):
                out[core * BPC + 4 * g + j, :, 0] = r[g, 32 * j:32 * j + K]
    return out


def kernel(timecaps, conv_w, conv_b, B_bias):
    in_maps = build_in_maps(timecaps, conv_w, conv_b, B_bias)
    nc = _get_program()
    res = run_bass_kernel_spmd(nc, in_maps, list(range(NCORES)))
    return assemble_out(res)


if __name__ == "__main__":
    rng = np.random.default_rng(0)
    ins = {
        "timecaps": rng.standard_normal((B_FULL, N, DT), dtype=np.float32),
        "conv_w": (rng.standard_normal((DT, 1, NO), dtype=np.float32) * 0.05),
        "conv_b": np.zeros((NO,), dtype=np.float32),
        "B_bias": (rng.standard_normal((K, 1, N), dtype=np.float32) * 0.05),
    }
    print(kernel(**ins)[:2, :, 0])


# revision 17
# speedup vs baseline: 1.3380x; 1.0127x over previous
"""Trainium2 Bass kernel for nn_Classifier (capsule conv + routing), v2.

Math (validated vs jax reference; fp16 operands give ~3e-3 rel err):
  W = conv_w[:,0,:]                                    # [16, 640]
  y[b,i,o]   = relu(sum_t x[b,i,t] W[t,o] + conv_b[o])
  U[b,k,i,d] = y[b,i,k*64+d]
  Usum[b,k,d]= sum_i U[b,k,i,d]
  logits     = (U . Usum)/4 -> softmax over i -> C;  Cb = C + B_bias
  S[b,k,:]   = sum_i Cb[b,k,i] U[b,k,i,:]
  out[b,k]   = n2/(n2+1),  n2 = |S|^2   (eps/sqrt factor ~1e-7, dropped)

Design notes (v2, ~3x faster than v1):
  - All matmul operands fp16 (1 col/cycle PE stream vs 2 for f32r; FWL
    weight loads; 16-bit DVE packing on SBUF ops). fp16 keeps 10 mantissa
    bits -> 3e-3 end-to-end vs 1.7e-2 for bf16 (softmax logits ~50-120).
  - Host pre-packs x into PE row-band layout and all constant masks, so
    the kernel has zero on-chip preamble (no iota/affine_select/PE input
    transposes): 2 input DMAs total.
  - Conv computed in both orientations on PE (contraction=17 row-banded
    4x via tile_position). PSUM->SBUF evictions (the real bottleneck:
    f32 PSUM reads run 1 elem/cycle) alternate DVE/ACT; relu and the
    usum row-reduction are fused into the eviction (accum_out).
  - gmat (usum-scaled logit weights) built on GpSimd (idle otherwise).
  - Single PSUM scope, 8 banks via tag reuse, so conv(g1) overlaps
    routing(g0) and the PE never idles long enough to lose HAM warmup.
  - Tail: out = n2/(n2+1) on [128,2], PE-transposed to [2,128] and
    stored with one 512B DMA (v1 used 8 scattered 40B DMAs, ~10us).

Per-core layout (8 batches/core, b = 4g+j, g in {0,1}, j in 0..3):
  xT[g]  [128,512] rows 32j+t = x[b,i,t], row 32j+16 = 1.0 (bias fold)
  w4     [128,640] rows 32j+t = W[t,o], row 32j+16 = conv_b
  yr_oi[b] [128,5,512]  chunk c: y[o=c*128+p, i]      (fp16)
  yr_io[g] [128,4,4,640] [p, j, q, o]: y[i=q*128+p, o] (fp16)
  logits via G matmul (G[o,k'] = 0.25*usum[o]*[class(o)==k']), softmax
  with per-row max, Cb = exp/Z + B; CbT via PE transpose; S col-tiled.
"""

import numpy as np

import concourse.bass as bass
import concourse.mybir as mybir
import concourse.tile as tile
from concourse import bacc
from concourse.bass_utils import run_bass_kernel_spmd

F32 = mybir.dt.float32
F16 = mybir.dt.float16
BF16 = mybir.dt.bfloat16

B_FULL = 64
N = 512          # num timecaps (routing dim i)
DT = 16          # dim timecaps (conv contraction)
K = 10           # classes
D = 64           # dim classes
NO = K * D       # 640 conv output channels
NCORES = 8
BPC = B_FULL // NCORES   # 8 batches per core

# const-block column offsets
C_XT0, C_XT1, C_W4, C_BB, C_GM, C_SM, C_ID = 0, 512, 1024, 1664, 2176, 2226, 2866
C_TOT = 2994


def _build_program():
    nc = bacc.Bacc("TRN2", target_bir_lowering=False)
    cst_in = nc.declare_dram_parameter("cst", [128, C_TOT], F16, isOutput=False)
    out_d = nc.declare_dram_parameter("out", [2, 128], F16, isOutput=True)

    AF = mybir.ActivationFunctionType
    OP = mybir.AluOpType

    with tile.TileContext(nc) as tc:
        with tc.tile_pool(name="const", bufs=1) as cpool:
            cst = cpool.tile([128, C_TOT], F16, name="cst", tag="cst")
            dummy = cpool.tile([128, 1], F32, name="dummy", tag="dummy")
            # load x+w first so conv can start; rest lands during conv
            nc.sync.dma_start(cst[:, 0:C_BB], cst_in[:, 0:C_BB])
            nc.sync.dma_start(cst[:, C_BB:C_TOT], cst_in[:, C_BB:C_TOT])
            # prefetch the exp table set (relu/copy ride along in every set)
            nc.vector.memset(dummy[:], 0.0)
            nc.scalar.activation(out=dummy[:], in_=dummy[:], func=AF.Exp)

            xT = [cst[:, C_XT0:C_XT0 + N], cst[:, C_XT1:C_XT1 + N]]
            w4 = cst[:, C_W4:C_W4 + NO]
            bb = cst[:, C_BB:C_BB + N]
            gm = cst[:, C_GM:C_GM + 5 * K]
            sm = cst[:, C_SM:C_SM + NO]
            idt = cst[:, C_ID:C_ID + 128]

            with tc.tile_pool(name="data", bufs=1) as dpool:
                yr_oi = [dpool.tile([128, 5, N], F16, name=f"yroi{b}", tag=f"yroi{b}")
                         for b in range(BPC)]
                yr_io = [dpool.tile([128, 4, 4, NO], BF16, name=f"yrio{g}", tag=f"yrio{g}")
                         for g in range(2)]
                usum = [dpool.tile([128, 4, 5], F32, name=f"us{g}", tag=f"us{g}")
                        for g in range(2)]
                gmat = [dpool.tile([128, 4, 5, K], F16, name=f"gmat{g}", tag=f"gmat{g}")
                        for g in range(2)]
                exp_sb = [dpool.tile([128, N], F16, name=f"exp{g}", tag=f"exp{g}")
                          for g in range(2)]
                cb_sb = [dpool.tile([128, N], F16, name=f"cb{g}", tag=f"cb{g}")
                         for g in range(2)]
                negmax = [dpool.tile([128, 1], F32, name=f"nm{g}", tag=f"nm{g}")
                          for g in range(2)]
                zsum = [dpool.tile([128, 1], F32, name=f"z{g}", tag=f"z{g}")
                        for g in range(2)]
                rz = [dpool.tile([128, 1], F32, name=f"rz{g}", tag=f"rz{g}")
                      for g in range(2)]
                ebt = [dpool.tile([128, 4, 4, K], BF16, name=f"ebt{g}", tag=f"ebt{g}")
                       for g in range(2)]
                smm = [dpool.tile([128, NO], F16, name=f"smm{g}", tag=f"smm{g}")
                       for g in range(2)]
                sqs = [dpool.tile([128, NO], F16, name=f"sqs{g}", tag=f"sqs{g}")
                       for g in range(2)]
                n2 = dpool.tile([128, 2], F32, name="n2", tag="n2")
                t_a = dpool.tile([128, 2], F32, name="t_a", tag="t_a")
                t_d = dpool.tile([128, 2], F32, name="t_d", tag="t_d")
                t_b = dpool.tile([128, 2], F16, name="t_b", tag="t_b")
                out_sb = dpool.tile([2, 128], F16, name="out_sb", tag="out_sb")

                evict_i = 0

                def evict(dst, src, acc=None):
                    # PSUM->SBUF relu eviction, alternating DVE/ACT
                    nonlocal evict_i
                    if evict_i % 2 == 0:
                        if acc is not None:
                            nc.vector.tensor_scalar(
                                out=dst, in0=src, scalar1=0.0, scalar2=0.0,
                                op0=OP.max, op1=OP.add, accum_out=acc)
                        else:
                            nc.vector.tensor_scalar(
                                out=dst, in0=src, scalar1=0.0, scalar2=None,
                                op0=OP.max)
                    else:
                        nc.scalar.activation(
                            out=dst, in_=src, func=AF.Relu, accum_out=acc)
                    evict_i += 1

                with tc.tile_pool(name="ps", bufs=1, space="PSUM") as pp:
                    lg = [None, None]
                    ps_sm = [None, None]
                    ps_sr = [None, None]

                    def conv_oi(g, cs):
                        for c in cs:
                            for j in range(4):
                                b = 4 * g + j
                                ps = pp.tile([128, N], F32, name="oi",
                                             tag="conv", bufs=5)
                                nc.tensor.matmul(
                                    ps[:],
                                    w4[32 * j:32 * j + DT + 1,
                                       c * 128:(c + 1) * 128],
                                    xT[g][32 * j:32 * j + DT + 1, :],
                                    start=True, stop=True,
                                    tile_position=(32 * j, 0))
                                evict(yr_oi[b][:, c, :], ps[:],
                                      usum[g][:, j, c:c + 1])
                                # G chunk on gpsimd (idle engine)
                                nc.gpsimd.tensor_scalar(
                                    out=gmat[g][:, j, c, :],
                                    in0=gm[:, c * K:(c + 1) * K],
                                    scalar1=usum[g][:, j, c:c + 1],
                                    scalar2=None, op0=OP.mult)

                    def conv_io(g, js):
                        for j in js:
                            b = 4 * g + j
                            rem = pp.tile([128, 4, 128], F32, name="rem",
                                          tag="rem", bufs=1)
                            for q in range(4):
                                ps = pp.tile([128, N], F32, name="iom",
                                             tag="conv", bufs=5)
                                nc.tensor.matmul(
                                    ps[:],
                                    xT[g][32 * j:32 * j + DT + 1,
                                          q * 128:(q + 1) * 128],
                                    w4[32 * j:32 * j + DT + 1, 0:N],
                                    start=True, stop=True,
                                    tile_position=(32 * j, 0))
                                nc.tensor.matmul(
                                    rem[:, q, :],
                                    xT[g][32 * j:32 * j + DT + 1,
                                          q * 128:(q + 1) * 128],
                                    w4[32 * j:32 * j + DT + 1, N:NO],
                                    start=True, stop=True,
                                    tile_position=(32 * j, 0))
                                evict(yr_io[g][:, j, q, 0:N], ps[:])
                            evict(yr_io[g][:, j, :, N:NO], rem[:])

                    def logits(g):
                        lg[g] = pp.tile([128, N], F32, name="lg",
                                        tag="lg", bufs=2)
                        for c in range(5):
                            for j in range(4):
                                b = 4 * g + j
                                nc.tensor.matmul(
                                    lg[g][32 * j:32 * j + K, :],
                                    gmat[g][:, j, c, :],
                                    yr_oi[b][:, c, :],
                                    start=(c == 0), stop=(c == 4),
                                    tile_position=(0, 32 * j))

                    def softmax(g):
                        nc.vector.tensor_reduce(
                            out=negmax[g][:], in_=lg[g][:],
                            op=OP.max, axis=mybir.AxisListType.X, negate=True)
                        nc.scalar.activation(
                            out=exp_sb[g][:], in_=lg[g][:], func=AF.Exp,
                            bias=negmax[g][:], scale=1.0,
                            accum_out=zsum[g][:])
                        nc.vector.reciprocal(rz[g][:], zsum[g][:])
                        nc.vector.scalar_tensor_tensor(
                            out=cb_sb[g][:], in0=exp_sb[g][:],
                            scalar=rz[g][:], in1=bb[:],
                            op0=OP.mult, op1=OP.add)

                    def cb_transpose(g):
                        cbt = pp.tile([128, 4, 128], F16, name="cbt",
                                      tag="lg", bufs=2)
                        for q in range(4):
                            nc.tensor.transpose(
                                cbt[:, q, :],
                                cb_sb[g][:, q * 128:(q + 1) * 128],
                                idt[:])
                        # transposed cols are 32j+k' -> pick each j's K cols
                        nc.vector.tensor_copy(
                            ebt[g][:],
                            cbt[:].rearrange("p q (a w) -> p q a w", a=4)[
                                :, :, :, 0:K])

                    def s_matmuls(g):
                        ps_sm[g] = pp.tile([128, N], F32, name="sm",
                                           tag="conv", bufs=5)
                        ps_sr[g] = pp.tile([128, 128], F32, name="sr",
                                           tag="rem", bufs=1)
                        for q in range(4):
                            for j in range(4):
                                nc.tensor.matmul(
                                    ps_sm[g][32 * j:32 * j + K, :],
                                    ebt[g][:, q, j, :],
                                    yr_io[g][:, j, q, 0:N],
                                    start=(q == 0), stop=(q == 3),
                                    tile_position=(0, 32 * j))
                                nc.tensor.matmul(
                                    ps_sr[g][32 * j:32 * j + K, :],
                                    ebt[g][:, q, j, :],
                                    yr_io[g][:, j, q, N:NO],
                                    start=(q == 0), stop=(q == 3),
                                    tile_position=(0, 32 * j))

                    def s_norm(g):
                        nc.vector.tensor_tensor(
                            out=smm[g][:, 0:N], in0=ps_sm[g][:],
                            in1=sm[:, 0:N], op=OP.mult)
                        nc.vector.tensor_tensor(
                            out=smm[g][:, N:NO], in0=ps_sr[g][:],
                            in1=sm[:, N:NO], op=OP.mult)
                        nc.vector.scalar_tensor_tensor(
                            out=sqs[g][:], in0=smm[g][:],
                            scalar=1.0, in1=smm[g][:],
                            op0=OP.mult, op1=OP.mult,
                            accum_out=n2[:, g:g + 1])

    # ---- phase schedule (engine queues are FIFO; order = schedule) ----
                    conv_oi(0, range(5))
                    logits(0)
                    conv_oi(1, [0, 1, 2])
                    softmax(0)
                    cb_transpose(0)
                    conv_oi(1, [3, 4])
                    conv_io(0, range(4))
                    conv_io(1, [0])
                    s_matmuls(0)
                    conv_io(1, [1])
                    logits(1)
                    s_norm(0)
                    conv_io(1, [2])
                    softmax(1)
                    cb_transpose(1)
                    conv_io(1, [3])
                    s_matmuls(1)
                    s_norm(1)

                    # ---- squash tail: out = n2/(n2+1), transposed store
                    nc.vector.tensor_scalar(
                        out=t_a[:], in0=n2[:], scalar1=1.0, scalar2=None,
                        op0=OP.add)
                    nc.vector.reciprocal(t_d[:], t_a[:])
                    nc.vector.tensor_tensor(
                        out=t_b[:], in0=n2[:], in1=t_d[:], op=OP.mult)
                    outT = pp.tile([2, 128], F16, name="outT",
                                   tag="lg", bufs=2)
                    nc.tensor.transpose(outT[:], t_b[:], idt[:])
                    nc.vector.tensor_copy(out_sb[:], outT[:])
                    nc.sync.dma_start(out_d[:, :], out_sb[:])
    nc.compile()
    return nc


_PROGRAM_CACHE = None


def _get_program():
    global _PROGRAM_CACHE
    if _PROGRAM_CACHE is None:
        _PROGRAM_CACHE = _build_program()
    return _PROGRAM_CACHE


def _build_const_common():
    """Constant part of the cst block (cols C_W4..end), batch-independent."""
    blk = np.zeros((128, C_TOT), dtype=np.float16)
    jj = np.arange(4)
    # gm: [p, c*10+k'] = 0.25 if k' == 2c + p//64
    p = np.arange(128)
    for c in range(5):
        for kp in range(K):
            blk[:, C_GM + c * K + kp] = np.where(2 * c + p // 64 == kp, 0.25, 0.0)
    # sm: rows 32j+k', cols [64k',64(k'+1)) = 1
    for j in range(4):
        for kp in range(K):
            blk[32 * j + kp, C_SM + D * kp:C_SM + D * (kp + 1)] = 1.0
    # identity
    blk[:, C_ID:C_ID + 128] = np.eye(128, dtype=np.float16)
    return blk


_CONST_COMMON = None


def build_in_maps(timecaps, conv_w, conv_b, B_bias):
    global _CONST_COMMON
    timecaps = np.asarray(timecaps, dtype=np.float32)
    conv_w = np.asarray(conv_w, dtype=np.float32)
    conv_b = np.asarray(conv_b, dtype=np.float32)
    B_bias = np.asarray(B_bias, dtype=np.float32)

    if _CONST_COMMON is None:
        _CONST_COMMON = _build_const_common()
    base = _CONST_COMMON.copy()
    W = conv_w[:, 0, :].astype(np.float16)         # [16, 640]
    cb16 = conv_b.astype(np.float16)
    bb16 = B_bias[:, 0, :].astype(np.float16)      # [10, 512]
    for j in range(4):
        base[32 * j:32 * j + DT, C_W4:C_W4 + NO] = W
        base[32 * j + DT, C_W4:C_W4 + NO] = cb16
        base[32 * j:32 * j + K, C_BB:C_BB + N] = bb16

    # x -> [core, g, j, t, i] fp16 row-band layout
    xt = timecaps.astype(np.float16).transpose(0, 2, 1)   # [64, 16, 512]
    xt = xt.reshape(NCORES, 2, 4, DT, N)

    in_maps = []
    for core in range(NCORES):
        cst = base.copy()
        for g in range(2):
            col = C_XT0 if g == 0 else C_XT1
            for j in range(4):
                cst[32 * j:32 * j + DT, col:col + N] = xt[core, g, j]
                cst[32 * j + DT, col:col + N] = 1.0
        in_maps.append({"cst": cst})
    return in_maps


def assemble_out(res):
    out = np.zeros((B_FULL, K, 1), dtype=np.float32)
    for core in range(NCORES):
        r = np.asarray(res.results[core]["out"], dtype=np.float32)  # [2, 128]
        for g in range(2):
            for j in range(4):
                out[core * BPC + 4 * g + j, :, 0] = r[g, 32 * j:32 * j + K]
    return out


def kernel(timecaps, conv_w, conv_b, B_bias):
    in_maps = build_in_maps(timecaps, conv_w, conv_b, B_bias)
    nc = _get_program()
    res = run_bass_kernel_spmd(nc, in_maps, list(range(NCORES)))
    return assemble_out(res)


if __name__ == "__main__":
    rng = np.random.default_rng(0)
    ins = {
        "timecaps": rng.standard_normal((B_FULL, N, DT), dtype=np.float32),
        "conv_w": (rng.standard_normal((DT, 1, NO), dtype=np.float32) * 0.05),
        "conv_b": np.zeros((NO,), dtype=np.float32),
        "B_bias": (rng.standard_normal((K, 1, N), dtype=np.float32) * 0.05),
    }
    print(kernel(**ins)[:2, :, 0])


# revision 18
# speedup vs baseline: 1.4257x; 1.0656x over previous
"""Trainium2 Bass kernel for nn_Classifier (capsule conv + routing), v2.

Math (validated vs jax reference; fp16 operands give ~3e-3 rel err):
  W = conv_w[:,0,:]                                    # [16, 640]
  y[b,i,o]   = relu(sum_t x[b,i,t] W[t,o] + conv_b[o])
  U[b,k,i,d] = y[b,i,k*64+d]
  Usum[b,k,d]= sum_i U[b,k,i,d]
  logits     = (U . Usum)/4 -> softmax over i -> C;  Cb = C + B_bias
  S[b,k,:]   = sum_i Cb[b,k,i] U[b,k,i,:]
  out[b,k]   = n2/(n2+1),  n2 = |S|^2   (eps/sqrt factor ~1e-7, dropped)

Design notes (v2, ~3x faster than v1):
  - All matmul operands fp16 (1 col/cycle PE stream vs 2 for f32r; FWL
    weight loads; 16-bit DVE packing on SBUF ops). fp16 keeps 10 mantissa
    bits -> 3e-3 end-to-end vs 1.7e-2 for bf16 (softmax logits ~50-120).
  - Host pre-packs x into PE row-band layout and all constant masks, so
    the kernel has zero on-chip preamble (no iota/affine_select/PE input
    transposes): 2 input DMAs total.
  - Conv computed in both orientations on PE (contraction=17 row-banded
    4x via tile_position). PSUM->SBUF evictions (the real bottleneck:
    f32 PSUM reads run 1 elem/cycle) alternate DVE/ACT; relu and the
    usum row-reduction are fused into the eviction (accum_out).
  - gmat (usum-scaled logit weights) built on GpSimd (idle otherwise).
  - Single PSUM scope, 8 banks via tag reuse, so conv(g1) overlaps
    routing(g0) and the PE never idles long enough to lose HAM warmup.
  - Tail: out = n2/(n2+1) on [128,2], PE-transposed to [2,128] and
    stored with one 512B DMA (v1 used 8 scattered 40B DMAs, ~10us).

Per-core layout (8 batches/core, b = 4g+j, g in {0,1}, j in 0..3):
  xT[g]  [128,512] rows 32j+t = x[b,i,t], row 32j+16 = 1.0 (bias fold)
  w4     [128,640] rows 32j+t = W[t,o], row 32j+16 = conv_b
  yr_oi[b] [128,5,512]  chunk c: y[o=c*128+p, i]      (fp16)
  yr_io[g] [128,4,4,640] [p, j, q, o]: y[i=q*128+p, o] (fp16)
  logits via G matmul (G[o,k'] = 0.25*usum[o]*[class(o)==k']), softmax
  with per-row max, Cb = exp/Z + B; CbT via PE transpose; S col-tiled.
"""

import numpy as np

import concourse.bass as bass
import concourse.mybir as mybir
import concourse.tile as tile
from concourse import bacc
from concourse.bass_utils import run_bass_kernel_spmd

F32 = mybir.dt.float32
F16 = mybir.dt.float16
BF16 = mybir.dt.bfloat16

B_FULL = 64
N = 512          # num timecaps (routing dim i)
DT = 16          # dim timecaps (conv contraction)
K = 10           # classes
D = 64           # dim classes
NO = K * D       # 640 conv output channels
NCORES = 8
BPC = B_FULL // NCORES   # 8 batches per core

# const-block column offsets
C_XT0, C_XT1, C_W4, C_BB, C_GM, C_SM, C_ID = 0, 512, 1024, 1664, 2176, 2226, 2866
C_TOT = 2994


def _build_program():
    nc = bacc.Bacc("TRN2", target_bir_lowering=False)
    cst_in = nc.declare_dram_parameter("cst", [128, C_TOT], F16, isOutput=False)
    out_d = nc.declare_dram_parameter("out", [2, 128], F16, isOutput=True)

    AF = mybir.ActivationFunctionType
    OP = mybir.AluOpType

    with tile.TileContext(nc) as tc:
        with tc.tile_pool(name="const", bufs=1) as cpool:
            cst = cpool.tile([128, C_TOT], F16, name="cst", tag="cst")
            dummy = cpool.tile([128, 1], F32, name="dummy", tag="dummy")
            # load x+w first so conv can start; rest lands during conv
            nc.sync.dma_start(cst[:, 0:C_BB], cst_in[:, 0:C_BB])
            nc.sync.dma_start(cst[:, C_BB:C_TOT], cst_in[:, C_BB:C_TOT])
            # prefetch the exp table set (relu/copy ride along in every set)
            nc.vector.memset(dummy[:], 0.0)
            nc.scalar.activation(out=dummy[:], in_=dummy[:], func=AF.Exp)

            xT = [cst[:, C_XT0:C_XT0 + N], cst[:, C_XT1:C_XT1 + N]]
            w4 = cst[:, C_W4:C_W4 + NO]
            bb = cst[:, C_BB:C_BB + N]
            gm = cst[:, C_GM:C_GM + 5 * K]
            sm = cst[:, C_SM:C_SM + NO]
            idt = cst[:, C_ID:C_ID + 128]

            with tc.tile_pool(name="data", bufs=1) as dpool:
                yr_oi = [dpool.tile([128, 5, N], F16, name=f"yroi{b}", tag=f"yroi{b}")
                         for b in range(BPC)]
                yr_io = [dpool.tile([128, 4, 4, NO], BF16, name=f"yrio{g}", tag=f"yrio{g}")
                         for g in range(2)]
                usum = [dpool.tile([128, 4, 5], F32, name=f"us{g}", tag=f"us{g}")
                        for g in range(2)]
                gmat = [dpool.tile([128, 4, 5, K], F16, name=f"gmat{g}", tag=f"gmat{g}")
                        for g in range(2)]
                exp_sb = [dpool.tile([128, N], F16, name=f"exp{g}", tag=f"exp{g}")
                          for g in range(2)]
                cb_sb = [dpool.tile([128, N], F16, name=f"cb{g}", tag=f"cb{g}")
                         for g in range(2)]
                negmax = [dpool.tile([128, 1], F32, name=f"nm{g}", tag=f"nm{g}")
                          for g in range(2)]
                zsum = [dpool.tile([128, 1], F32, name=f"z{g}", tag=f"z{g}")
                        for g in range(2)]
                rz = [dpool.tile([128, 1], F32, name=f"rz{g}", tag=f"rz{g}")
                      for g in range(2)]
                ebt = [dpool.tile([128, 4, 4, K], BF16, name=f"ebt{g}", tag=f"ebt{g}")
                       for g in range(2)]
                smm = [dpool.tile([128, NO], F16, name=f"smm{g}", tag=f"smm{g}")
                       for g in range(2)]
                sqs = [dpool.tile([128, NO], F16, name=f"sqs{g}", tag=f"sqs{g}")
                       for g in range(2)]
                n2 = dpool.tile([128, 2], F32, name="n2", tag="n2")
                t_a = dpool.tile([128, 2], F32, name="t_a", tag="t_a")
                t_d = dpool.tile([128, 2], F32, name="t_d", tag="t_d")
                t_b = dpool.tile([128, 2], F16, name="t_b", tag="t_b")
                out_sb = dpool.tile([2, 128], F16, name="out_sb", tag="out_sb")

                evict_i = 0

                def evict(dst, src, acc=None):
                    # PSUM->SBUF relu eviction, alternating DVE/ACT
                    nonlocal evict_i
                    if evict_i % 2 == 0:
                        if acc is not None:
                            nc.vector.tensor_scalar(
                                out=dst, in0=src, scalar1=0.0, scalar2=0.0,
                                op0=OP.max, op1=OP.add, accum_out=acc)
                        else:
                            nc.vector.tensor_scalar(
                                out=dst, in0=src, scalar1=0.0, scalar2=None,
                                op0=OP.max)
                    else:
                        nc.scalar.activation(
                            out=dst, in_=src, func=AF.Relu, accum_out=acc)
                    evict_i += 1

                with tc.tile_pool(name="ps", bufs=1, space="PSUM") as pp:
                    lg = [None, None]
                    ps_sm = [None, None]
                    ps_sr = [None, None]

                    def conv_oi(g, cs):
                        for c in cs:
                            for j in range(4):
                                b = 4 * g + j
                                ps = pp.tile([128, N], F32, name="oi",
                                             tag="conv", bufs=5)
                                nc.tensor.matmul(
                                    ps[:],
                                    w4[32 * j:32 * j + DT + 1,
                                       c * 128:(c + 1) * 128],
                                    xT[g][32 * j:32 * j + DT + 1, :],
                                    start=True, stop=True,
                                    tile_position=(32 * j, 0))
                                evict(yr_oi[b][:, c, :], ps[:],
                                      usum[g][:, j, c:c + 1])
                                # G chunk on gpsimd (idle engine)
                                nc.gpsimd.tensor_scalar(
                                    out=gmat[g][:, j, c, :],
                                    in0=gm[:, c * K:(c + 1) * K],
                                    scalar1=usum[g][:, j, c:c + 1],
                                    scalar2=None, op0=OP.mult)

                    def conv_io(g, js):
                        for j in js:
                            b = 4 * g + j
                            rem = pp.tile([128, 4, 128], F32, name="rem",
                                          tag="rem", bufs=1)
                            for q in range(4):
                                ps = pp.tile([128, N], F32, name="iom",
                                             tag="conv", bufs=5)
                                nc.tensor.matmul(
                                    ps[:],
                                    xT[g][32 * j:32 * j + DT + 1,
                                          q * 128:(q + 1) * 128],
                                    w4[32 * j:32 * j + DT + 1, 0:N],
                                    start=True, stop=True,
                                    tile_position=(32 * j, 0))
                                nc.tensor.matmul(
                                    rem[:, q, :],
                                    xT[g][32 * j:32 * j + DT + 1,
                                          q * 128:(q + 1) * 128],
                                    w4[32 * j:32 * j + DT + 1, N:NO],
                                    start=True, stop=True,
                                    tile_position=(32 * j, 0))
                                evict(yr_io[g][:, j, q, 0:N], ps[:])
                            evict(yr_io[g][:, j, :, N:NO], rem[:])

                    def logits(g):
                        lg[g] = pp.tile([128, N], F32, name="lg",
                                        tag="lg", bufs=2)
                        for c in range(5):
                            for j in range(4):
                                b = 4 * g + j
                                nc.tensor.matmul(
                                    lg[g][32 * j:32 * j + K, :],
                                    gmat[g][:, j, c, :],
                                    yr_oi[b][:, c, :],
                                    start=(c == 0), stop=(c == 4),
                                    tile_position=(0, 32 * j))

                    def softmax(g):
                        nc.vector.tensor_reduce(
                            out=negmax[g][:], in_=lg[g][:],
                            op=OP.max, axis=mybir.AxisListType.X, negate=True)
                        nc.scalar.activation(
                            out=exp_sb[g][:], in_=lg[g][:], func=AF.Exp,
                            bias=negmax[g][:], scale=1.0,
                            accum_out=zsum[g][:])
                        nc.vector.reciprocal(rz[g][:], zsum[g][:])
                        nc.vector.scalar_tensor_tensor(
                            out=cb_sb[g][:], in0=exp_sb[g][:],
                            scalar=rz[g][:], in1=bb[:],
                            op0=OP.mult, op1=OP.add)

                    def cb_transpose(g):
                        cbt = pp.tile([128, 4, 128], F16, name="cbt",
                                      tag="lg", bufs=2)
                        for q in range(4):
                            nc.tensor.transpose(
                                cbt[:, q, :],
                                cb_sb[g][:, q * 128:(q + 1) * 128],
                                idt[:])
                        # transposed cols are 32j+k' -> pick each j's K cols
                        nc.vector.tensor_copy(
                            ebt[g][:],
                            cbt[:].rearrange("p q (a w) -> p q a w", a=4)[
                                :, :, :, 0:K])

                    def s_matmuls(g):
                        ps_sm[g] = pp.tile([128, N], F32, name="sm",
                                           tag="conv", bufs=5)
                        ps_sr[g] = pp.tile([128, 128], F32, name="sr",
                                           tag="rem", bufs=1)
                        for q in range(4):
                            for j in range(4):
                                nc.tensor.matmul(
                                    ps_sm[g][32 * j:32 * j + K, :],
                                    ebt[g][:, q, j, :],
                                    yr_io[g][:, j, q, 0:N],
                                    start=(q == 0), stop=(q == 3),
                                    tile_position=(0, 32 * j))
                                nc.tensor.matmul(
                                    ps_sr[g][32 * j:32 * j + K, :],
                                    ebt[g][:, q, j, :],
                                    yr_io[g][:, j, q, N:NO],
                                    start=(q == 0), stop=(q == 3),
                                    tile_position=(0, 32 * j))

                    def s_norm(g):
                        nc.vector.tensor_tensor(
                            out=smm[g][:, 0:N], in0=ps_sm[g][:],
                            in1=sm[:, 0:N], op=OP.mult)
                        nc.vector.tensor_tensor(
                            out=smm[g][:, N:NO], in0=ps_sr[g][:],
                            in1=sm[:, N:NO], op=OP.mult)
                        nc.vector.scalar_tensor_tensor(
                            out=sqs[g][:], in0=smm[g][:],
                            scalar=1.0, in1=smm[g][:],
                            op0=OP.mult, op1=OP.mult,
                            accum_out=n2[:, g:g + 1])

    # ---- phase schedule (engine queues are FIFO; order = schedule) ----
                    conv_oi(0, range(5))
                    logits(0)
                    conv_oi(1, [0, 1, 2])
                    softmax(0)
                    cb_transpose(0)
                    conv_oi(1, [3, 4])
                    conv_io(0, [0, 1])
                    logits(1)
                    conv_io(0, [2, 3])
                    softmax(1)
                    cb_transpose(1)
                    conv_io(1, [0, 1])
                    s_matmuls(0)
                    s_norm(0)
                    conv_io(1, [2, 3])
                    s_matmuls(1)
                    s_norm(1)

                    # ---- squash tail: out = n2/(n2+1), transposed store
                    nc.vector.tensor_scalar(
                        out=t_a[:], in0=n2[:], scalar1=1.0, scalar2=None,
                        op0=OP.add)
                    nc.vector.reciprocal(t_d[:], t_a[:])
                    nc.vector.tensor_tensor(
                        out=t_b[:], in0=n2[:], in1=t_d[:], op=OP.mult)
                    outT = pp.tile([2, 128], F16, name="outT",
                                   tag="lg", bufs=2)
                    nc.tensor.transpose(outT[:], t_b[:], idt[:])
                    nc.vector.tensor_copy(out_sb[:], outT[:])
                    nc.sync.dma_start(out_d[:, :], out_sb[:])
    nc.compile()
    return nc


_PROGRAM_CACHE = None


def _get_program():
    global _PROGRAM_CACHE
    if _PROGRAM_CACHE is None:
        _PROGRAM_CACHE = _build_program()
    return _PROGRAM_CACHE


def _build_const_common():
    """Constant part of the cst block (cols C_W4..end), batch-independent."""
    blk = np.zeros((128, C_TOT), dtype=np.float16)
    jj = np.arange(4)
    # gm: [p, c*10+k'] = 0.25 if k' == 2c + p//64
    p = np.arange(128)
    for c in range(5):
        for kp in range(K):
            blk[:, C_GM + c * K + kp] = np.where(2 * c + p // 64 == kp, 0.25, 0.0)
    # sm: rows 32j+k', cols [64k',64(k'+1)) = 1
    for j in range(4):
        for kp in range(K):
            blk[32 * j + kp, C_SM + D * kp:C_SM + D * (kp + 1)] = 1.0
    # identity
    blk[:, C_ID:C_ID + 128] = np.eye(128, dtype=np.float16)
    return blk


_CONST_COMMON = None


def build_in_maps(timecaps, conv_w, conv_b, B_bias):
    global _CONST_COMMON
    timecaps = np.asarray(timecaps, dtype=np.float32)
    conv_w = np.asarray(conv_w, dtype=np.float32)
    conv_b = np.asarray(conv_b, dtype=np.float32)
    B_bias = np.asarray(B_bias, dtype=np.float32)

    if _CONST_COMMON is None:
        _CONST_COMMON = _build_const_common()
    base = _CONST_COMMON.copy()
    W = conv_w[:, 0, :].astype(np.float16)         # [16, 640]
    cb16 = conv_b.astype(np.float16)
    bb16 = B_bias[:, 0, :].astype(np.float16)      # [10, 512]
    for j in range(4):
        base[32 * j:32 * j + DT, C_W4:C_W4 + NO] = W
        base[32 * j + DT, C_W4:C_W4 + NO] = cb16
        base[32 * j:32 * j + K, C_BB:C_BB + N] = bb16

    # x -> [core, g, j, t, i] fp16 row-band layout
    xt = timecaps.astype(np.float16).transpose(0, 2, 1)   # [64, 16, 512]
    xt = xt.reshape(NCORES, 2, 4, DT, N)

    in_maps = []
    for core in range(NCORES):
        cst = base.copy()
        for g in range(2):
            col = C_XT0 if g == 0 else C_XT1
            for j in range(4):
                cst[32 * j:32 * j + DT, col:col + N] = xt[core, g, j]
                cst[32 * j + DT, col:col + N] = 1.0
        in_maps.append({"cst": cst})
    return in_maps


def assemble_out(res):
    out = np.zeros((B_FULL, K, 1), dtype=np.float32)
    for core in range(NCORES):
        r = np.asarray(res.results[core]["out"], dtype=np.float32)  # [2, 128]
        for g in range(2):
            for j in range(4):
                out[core * BPC + 4 * g + j, :, 0] = r[g, 32 * j:32 * j + K]
    return out


def kernel(timecaps, conv_w, conv_b, B_bias):
    in_maps = build_in_maps(timecaps, conv_w, conv_b, B_bias)
    nc = _get_program()
    res = run_bass_kernel_spmd(nc, in_maps, list(range(NCORES)))
    return assemble_out(res)


if __name__ == "__main__":
    rng = np.random.default_rng(0)
    ins = {
        "timecaps": rng.standard_normal((B_FULL, N, DT), dtype=np.float32),
        "conv_w": (rng.standard_normal((DT, 1, NO), dtype=np.float32) * 0.05),
        "conv_b": np.zeros((NO,), dtype=np.float32),
        "B_bias": (rng.standard_normal((K, 1, N), dtype=np.float32) * 0.05),
    }
    print(kernel(**ins)[:2, :, 0])
